# revision 9
# baseline (speedup 1.0000x reference)
"""Trainium2 Bass kernel for DressedQuantumCircuit (12 qubits, 6 layers).

Strategy:
  - The quantum circuit is linear in the statevector.  Layer-1 RY gates are
    folded into the input angles (RY(t)RY(a) = RY(t+a)); the remaining
    circuit (CNOT chain + layers 2..6) is an input-independent orthogonal
    operator U' in R^{4096x4096}, precomputed on the host from `weights`.
  - The readout contracts to a single vector: out[b] = sum_j g[j] *
    (U' s0[b])_j^2 + post_b with g[j] = sum_w post_w[w] * (1-2*bit_w(j)).
  - Data-parallel over batch: 8192 samples -> 8 cores x 1024.
  - On device per core: transpose x, small pre-matmul -> quarter angles,
    ACT sin/cos + two double-angle steps (ACT Sin is only accurate for
    |x| <~ 2), product-state build via log-doubling tensor_scalar ops,
    PE-transpose into [state, batch] layout, then one 4096x4096 @
    4096x1024 matmul in float32r (TF32, full PE rate) streaming U tiles
    from HBM, fused square/weight/accumulate epilogue, ones-matmul
    partition reduction.
"""

import hashlib

import numpy as np

N_QUBITS = 12
N_LAYERS = 6
D = 4096                 # 2**N_QUBITS
D_IN = 512
BATCH = 8192
N_CORES = 8
B_CORE = BATCH // N_CORES      # 1024
N_CHUNKS = B_CORE // 128       # 8
K_TILES = D // 128             # 32
J_TILES = D // 128             # 32
N_HALF = B_CORE // 512         # halves of the per-core batch for N=512 mms

_prog_cache = {}
_ut_cache = {}


# ----------------------------------------------------------------- host math
def _build_ut(weights):
    """Simulate CNOT-chain + layers 2..6 on the identity.  Row i of the
    result is circuit(e_i), i.e. result = U'^T, exactly the [i, j] layout
    the device matmul consumes (contraction over i on partitions)."""
    key = hashlib.sha256(np.ascontiguousarray(weights)).hexdigest()
    if key in _ut_cache:
        return _ut_cache[key]
    N = N_QUBITS
    st = np.eye(D, dtype=np.float32)

    def ry_layer(st, thetas):
        for w in range(N):
            c = np.float32(np.cos(thetas[w] / 2))
            s = np.float32(np.sin(thetas[w] / 2))
            lo = 2 ** (N - 1 - w)
            sh = st.reshape(D, -1, 2, lo)
            a = sh[:, :, 0, :].copy()
            b = sh[:, :, 1, :]
            sh[:, :, 0, :] = c * a - s * b
            sh[:, :, 1, :] = s * a + c * b
        return st

    def cnot_chain(st):
        for w in range(N - 1):
            lt = 2 ** (N - 2 - w)
            sh = st.reshape(D, -1, 2, 1, 2, lt)
            a = sh[:, :, 1, :, 0, :].copy()
            sh[:, :, 1, :, 0, :] = sh[:, :, 1, :, 1, :]
            sh[:, :, 1, :, 1, :] = a
        return st

    wts = np.asarray(weights, dtype=np.float64)
    st = cnot_chain(st)
    for layer in range(1, N_LAYERS):
        st = ry_layer(st, wts[layer])
        st = cnot_chain(st)
    _ut_cache.clear()
    _ut_cache[key] = st
    return st


def _host_constants(pre_w, pre_b, weights, post_w, post_b):
    N = N_QUBITS
    wts = np.asarray(weights, dtype=np.float64)
    # quarter angle: a = x @ (pre_w.T * pi/16) + bias_a,  G = 4a,
    # v0 = cos(G), v1 = sin(G)
    wf = (np.asarray(pre_w, dtype=np.float64).T * (np.pi / 16.0))  # [512, 12]
    bias_a = (np.asarray(pre_b, dtype=np.float64) * (np.pi / 4.0)
              + wts[0] / 2.0 + np.pi / 4.0) / 4.0
    # pack wf as [128, 4*12]: wf_packed[p, t*12+w] = wf[t*128+p, w]
    wf_packed = np.ascontiguousarray(
        wf.reshape(4, 128, N).transpose(1, 0, 2).reshape(128, 4 * N)
    ).astype(np.float32)
    bias_sa = bias_a.reshape(N, 1).astype(np.float32)
    bias_ca = (bias_a + np.pi / 2.0).reshape(N, 1).astype(np.float32)
    # readout vector g
    j = np.arange(D)
    g = np.zeros(D, dtype=np.float64)
    for w in range(N):
        g += float(np.asarray(post_w).reshape(-1)[w]) * (
            1.0 - 2.0 * ((j >> (N - 1 - w)) & 1))
    g_packed = np.ascontiguousarray(
        g.reshape(K_TILES, 128).T).astype(np.float32)   # [128, 32]
    pb = np.asarray(post_b, dtype=np.float32).reshape(1, 1)
    ut = _build_ut(weights)
    return wf_packed, bias_sa, bias_ca, g_packed, pb, ut


# ------------------------------------------------------------- device program
def _build_program():
    import concourse.bass as bass
    import concourse.mybir as mybir
    import concourse.tile as tile
    from concourse import bacc
    from concourse.masks import make_identity

    f32 = mybir.dt.float32
    f32r = mybir.dt.float32r
    AF = mybir.ActivationFunctionType
    N = N_QUBITS

    nc = bacc.Bacc("TRN2", target_bir_lowering=False, debug=False,
                   num_devices=N_CORES)
    x_d = nc.dram_tensor("x", [B_CORE, D_IN], f32, kind="ExternalInput").ap()
    wf_d = nc.dram_tensor("wf", [128, 4 * N], f32, kind="ExternalInput").ap()
    bsa_d = nc.dram_tensor("bsa", [N, 1], f32, kind="ExternalInput").ap()
    bca_d = nc.dram_tensor("bca", [N, 1], f32, kind="ExternalInput").ap()
    g_d = nc.dram_tensor("g", [128, K_TILES], f32, kind="ExternalInput").ap()
    pb_d = nc.dram_tensor("pb", [1, 1], f32, kind="ExternalInput").ap()
    ut_d = nc.dram_tensor("ut", [D, D], f32r, kind="ExternalInput").ap()
    out_d = nc.dram_tensor("out", [1, B_CORE], f32, kind="ExternalOutput").ap()

    with tile.TileContext(nc) as tc:
        with (
            tc.tile_pool(name="const", bufs=1) as constp,
            tc.tile_pool(name="xt", bufs=2) as xtp,
            tc.tile_pool(name="xT", bufs=2) as xTp,
            tc.tile_pool(name="v", bufs=2) as vp,
            tc.tile_pool(name="dbl", bufs=1) as dblp,
            tc.tile_pool(name="s0", bufs=1) as s0p,
            tc.tile_pool(name="utp", bufs=10) as utp,
            tc.tile_pool(name="ep", bufs=3) as epp,
            tc.tile_pool(name="acc", bufs=1) as accp,
            tc.tile_pool(name="trps", bufs=2, space="PSUM") as tr_ps,
            tc.tile_pool(name="preps", bufs=1, space="PSUM") as pre_ps,
            tc.tile_pool(name="mmps", bufs=2, space="PSUM") as mm_ps,
            tc.tile_pool(name="finps", bufs=1, space="PSUM") as fin_ps,
        ):
            ident = constp.tile([128, 128], f32)
            make_identity(nc, ident[:])
            wf_sb = constp.tile([128, 4 * N], f32)
            nc.sync.dma_start(wf_sb[:], wf_d[:])
            bsa = constp.tile([N, 1], f32)
            nc.sync.dma_start(bsa[:], bsa_d[:])
            bca = constp.tile([N, 1], f32)
            nc.sync.dma_start(bca[:], bca_d[:])
            g_sb = constp.tile([128, K_TILES], f32)
            nc.sync.dma_start(g_sb[:], g_d[:])
            pb_sb = constp.tile([1, 1], f32)
            nc.sync.dma_start(pb_sb[:], pb_d[:])
            ones = constp.tile([128, 1], f32)
            nc.gpsimd.memset(ones[:], 1.0)

            # full initial state, [state i on 32 partition-tiles, batch]:
            # s0sb[p, t*B_CORE + b] = s0[t*128+p, b]
            s0sb = s0p.tile([128, K_TILES * B_CORE], f32r)

            for cc in range(N_CHUNKS):
                # ---- load + transpose x chunk -> xT [512d, 128b]
                xt = xtp.tile([128, D_IN], f32)
                nc.sync.dma_start(xt[:], x_d[cc * 128:(cc + 1) * 128, :])
                xT = xTp.tile([128, 4 * 128], f32)
                for k in range(4):
                    tp = tr_ps.tile([128, 128], f32)
                    nc.tensor.transpose(tp[:], xt[:, k * 128:(k + 1) * 128],
                                        ident[:])
                    nc.vector.tensor_copy(xT[:, k * 128:(k + 1) * 128], tp[:])
                # ---- pre-matmul: quarter angles a [12, 128b] (psum)
                pre = pre_ps.tile([N, 128], f32)
                for k in range(4):
                    nc.tensor.matmul(pre[:], wf_sb[:, k * N:(k + 1) * N],
                                     xT[:, k * 128:(k + 1) * 128],
                                     start=(k == 0), stop=(k == 3))
                # ---- sin/cos of quarter angle, two double-angle steps
                sa = vp.tile([N, 128], f32, tag="sa")
                ca = vp.tile([N, 128], f32, tag="ca")
                nc.scalar.activation(sa[:], pre[:], AF.Sin, bias=bsa[:])
                nc.scalar.activation(ca[:], pre[:], AF.Sin, bias=bca[:])
                t1 = vp.tile([N, 128], f32, tag="t1")
                u1 = vp.tile([N, 128], f32, tag="u1")
                s2 = vp.tile([N, 128], f32, tag="s2")
                c2 = vp.tile([N, 128], f32, tag="c2")
                nc.vector.tensor_mul(t1[:], sa[:], ca[:])
                nc.vector.tensor_add(s2[:], t1[:], t1[:])
                nc.vector.tensor_mul(u1[:], sa[:], sa[:])
                nc.scalar.activation(c2[:], u1[:], AF.Copy, bias=1.0,
                                     scale=-2.0)
                # vt rows 0..11 = v0 = cos(G), rows 32..43 = v1 = sin(G)
                # (v1 base partition must be 32-aligned for DVE writes)
                vt = vp.tile([32 + N, 128], f32, tag="vt")
                t2 = vp.tile([N, 128], f32, tag="t2")
                u2 = vp.tile([N, 128], f32, tag="u2")
                nc.vector.tensor_mul(t2[:], s2[:], c2[:])
                nc.vector.tensor_add(vt[32:32 + N, :], t2[:], t2[:])
                nc.vector.tensor_mul(u2[:], s2[:], s2[:])
                nc.scalar.activation(vt[0:N, :], u2[:], AF.Copy, bias=1.0,
                                     scale=-2.0)
                # transpose vt -> vT [128b, 44]: v0[w]=col w, v1[w]=col 32+w
                vtp = tr_ps.tile([128, 32 + N], f32, tag="tp", name="vtp")
                nc.tensor.transpose(vtp[:], vt[:], ident[0:32 + N, 0:32 + N])
                vT = vp.tile([128, 32 + N], f32, tag="vT")
                nc.vector.tensor_copy(vT[:], vtp[:])
                # ---- product state build (free-dim log-doubling):
                # s0T[b, i], i bits: wire0 = MSB ... wire11 = LSB
                pA = dblp.tile([128, D // 2], f32, tag="pA")
                pB = dblp.tile([128, D], f32, tag="pB")
                nc.vector.tensor_copy(pA[:, 0:1], vT[:, N - 1:N])
                nc.vector.tensor_copy(pA[:, 1:2], vT[:, 32 + N - 1:32 + N])
                cur, nxt = pA, pB
                L = 2
                for w in range(N - 2, -1, -1):
                    nc.vector.tensor_scalar_mul(nxt[:, 0:L], cur[:, 0:L],
                                                vT[:, w:w + 1])
                    nc.vector.tensor_scalar_mul(nxt[:, L:2 * L], cur[:, 0:L],
                                                vT[:, 32 + w:32 + w + 1])
                    cur, nxt = nxt, cur
                    L *= 2
                # ---- transpose into s0sb [i, b] (f32r rounding on copy)
                for t in range(K_TILES):
                    tp = tr_ps.tile([128, 128], f32)
                    nc.tensor.transpose(tp[:], cur[:, t * 128:(t + 1) * 128],
                                        ident[:])
                    nc.vector.tensor_copy(
                        s0sb[:, t * B_CORE + cc * 128:
                             t * B_CORE + (cc + 1) * 128], tp[:])

            # ---- main matmul: out_state[j, b] = sum_i Ut[i, j] s0[i, b]
            acc = accp.tile([128, B_CORE], f32)
            for j in range(J_TILES):
                ps = [mm_ps.tile([128, 512], f32, tag=f"mm{h}",
                                 name=f"ps{h}_{j}")
                      for h in range(N_HALF)]
                for k in range(K_TILES):
                    utt = utp.tile([128, 128], f32r)
                    nc.sync.dma_start(
                        utt[:], ut_d[k * 128:(k + 1) * 128,
                                     j * 128:(j + 1) * 128])
                    for h in range(N_HALF):
                        nc.tensor.matmul(
                            ps[h][:], utt[:],
                            s0sb[:, k * B_CORE + h * 512:
                                 k * B_CORE + (h + 1) * 512],
                            start=(k == 0), stop=(k == K_TILES - 1))
                # epilogue: acc[:, h] (+)= psum^2 * g[:, j]
                for h in range(N_HALF):
                    sq = epp.tile([128, 512], f32, tag="sq")
                    nc.scalar.activation(sq[:], ps[h][:], AF.Square)
                    dst = acc[:, h * 512:(h + 1) * 512]
                    if j == 0:
                        nc.vector.tensor_scalar_mul(dst, sq[:],
                                                    g_sb[:, j:j + 1])
                    else:
                        nc.vector.tensor_scalar_mul(sq[:], sq[:],
                                                    g_sb[:, j:j + 1])
                        nc.vector.tensor_add(dst, dst, sq[:])
            # ---- partition reduction + post_b bias
            out_sb = accp.tile([1, B_CORE], f32)
            for h in range(N_HALF):
                fin = fin_ps.tile([1, 512], f32)
                nc.tensor.matmul(fin[:], ones[:],
                                 acc[:, h * 512:(h + 1) * 512],
                                 start=True, stop=True)
                nc.scalar.activation(out_sb[:, h * 512:(h + 1) * 512], fin[:],
                                     AF.Identity, bias=pb_sb[:])
            nc.sync.dma_start(out_d[:], out_sb[:])

    nc.compile()
    return nc


# ------------------------------------------------------------------- entry
def kernel(x, pre_w, pre_b, weights, post_w, post_b):
    from concourse import bass_utils

    x = np.ascontiguousarray(np.asarray(x, dtype=np.float32))
    wf_packed, bias_sa, bias_ca, g_packed, pb, ut = _host_constants(
        pre_w, pre_b, weights, post_w, post_b)

    if "nc" not in _prog_cache:
        _prog_cache["nc"] = _build_program()
    nc = _prog_cache["nc"]

    in_maps = []
    for c in range(N_CORES):
        in_maps.append({
            "x": x[c * B_CORE:(c + 1) * B_CORE],
            "wf": wf_packed, "bsa": bias_sa, "bca": bias_ca,
            "g": g_packed, "pb": pb, "ut": ut,
        })
    res = bass_utils.run_bass_kernel_spmd(nc, in_maps,
                                          core_ids=list(range(N_CORES)))
    out = np.concatenate([r["out"][0] for r in res.results])
    return out.reshape(BATCH, 1).astype(np.float32)
